# revision 1
# baseline (speedup 1.0000x reference)
"""Causal self-attention (B=4, T=2048, C=1024, 16 heads) on 8 NeuronCores.

Sharding: core c -> batch b=c//2, head group g=c%2 (8 heads each).
Each core computes q,k,v for its 8 heads, causal flash-style attention,
and a partial output projection (row-slice of w_proj). Host sums the two
partials per batch and adds b_proj.

Layout strategy (all matmul inputs written as float32r for full PE speed):
  - x is pre-transposed on host -> xT [C, T] so the C-contraction sits on
    SBUF partitions for the qkv projections.
  - q, k are produced transposed (qT/kT [head_dim, T]) which is exactly the
    operand layout for S^T = k q^T tiles [Tk, Tq]; softmax runs along the
    free (Tq) axis per k-row, with row sums obtained for free by appending
    a ones-column to v in the P@V matmul (out row 64 = sum of P columns).
  - v is produced in natural [T, 512] layout (x as stationary operand) and
    stored interleaved per-head with the ones column: [T, 8*65].
  - Normalization: the sums row is broadcast across 64 partitions with a
    K=1 ones matmul, reciprocal'd on DVE, and multiplied into y^T tiles.
  - proj: out[t,c] accumulated over 4 ydim k-tiles (= head pairs).
"""
import numpy as np
import concourse.bass as bass
from concourse import bacc
import concourse.tile as tile
import concourse.mybir as mybir
from concourse.bass_utils import run_bass_kernel_spmd

B, T, C = 4, 2048, 1024
HD = 64            # head dim
HL = 8             # local heads per core
PAIRS = 4          # local head pairs
KT = C // 128      # 8 contraction tiles for qkv
TT = T // 128      # 16 row tiles of T
NQ = T // 512      # 4 query chunks of 512
F32 = mybir.dt.float32
F32R = mybir.dt.float32r
EXP = mybir.ActivationFunctionType.Exp

_NC_CACHE = {}


def _build(nrep=1, bias=False):
    nc = bacc.Bacc("TRN2", target_bir_lowering=False, debug=False)
    xT_d = nc.dram_tensor("xT", [C, T], F32, kind="ExternalInput")
    wqkv_d = nc.dram_tensor("wqkv", [C, 1536], F32, kind="ExternalInput")
    bqkv_d = nc.dram_tensor("bqkv", [1536], F32, kind="ExternalInput")
    wp_d = nc.dram_tensor("wp", [512, C], F32, kind="ExternalInput")
    out_d = nc.dram_tensor("out", [T, C], F32, kind="ExternalOutput")

    # constants baked into the NEFF
    import ml_dtypes
    masks_np = np.zeros((4, 128, 512), dtype=np.float32)
    for j in range(4):
        for p in range(128):
            masks_np[j, p, 128 * j + p:] = 1.0
    masks_d = nc.inline_tensor(masks_np.astype(ml_dtypes.bfloat16), name="cmasks")
    ones_r_d = nc.inline_tensor(np.ones((1, 128), dtype=np.float32), name="ones_r")
    ones_v_d = nc.inline_tensor(np.ones((128, 8), dtype=np.float32), name="ones_v")
    # [2,128] selector: row0 -> partitions 0-63, row1 -> partitions 64-127
    sel_np = np.zeros((2, 128), dtype=np.float32)
    sel_np[0, 0:64] = 1.0
    sel_np[1, 64:128] = 1.0
    sel_d = nc.inline_tensor(sel_np, name="sel2")

    with tile.TileContext(nc) as tc:
        with (
            tc.tile_pool(name="xt", bufs=8) as p_xt,        # xT tiles, later wp tiles
            tc.tile_pool(name="w", bufs=8) as p_w,          # w_v then per-pair w_q/w_k
            tc.tile_pool(name="vaug", bufs=16) as p_va,
            tc.tile_pool(name="qk", bufs=2) as p_qk,
            tc.tile_pool(name="yt", bufs=4) as p_yt,
            tc.tile_pool(name="mask", bufs=4) as p_mask,
            tc.tile_pool(name="pexp", bufs=3) as p_px,
            tc.tile_pool(name="tiny", bufs=2) as p_tiny,
            tc.tile_pool(name="mm", bufs=2, space="PSUM") as pp_mm,
            tc.tile_pool(name="st", bufs=2, space="PSUM") as pp_st,
            tc.tile_pool(name="ot", bufs=2, space="PSUM") as pp_ot,
        ):
            # ---- constants ----
            ones_r = p_tiny.tile([1, 128], F32R, tag="onesr")
            nc.sync.dma_start(out=ones_r[:], in_=ones_r_d.ap().bitcast(F32R))
            # ones row living at partition 64 (for K-dim alignment with s rows)
            ones64 = p_tiny.tile([65, 128], F32R, tag="ones64")
            nc.sync.dma_start(out=ones64[64:65, :], in_=ones_r_d.ap().bitcast(F32R))
            sel2 = p_tiny.tile([2, 128], F32R, tag="sel2")
            nc.sync.dma_start(out=sel2[:], in_=sel_d.ap().bitcast(F32R))
            masks = []
            for j in range(4):
                m = p_mask.tile([128, 512], mybir.dt.bfloat16, tag="mask",
                                name=f"mask{j}")
                nc.sync.dma_start(out=m[:], in_=masks_d.ap()[j])
                masks.append(m)

            for rep in range(nrep):
                # ---- load xT ----
                xts = []
                for k in range(KT):
                    t_ = p_xt.tile([128, T], F32R, tag="xt", name=f"xt{rep}_{k}")
                    nc.sync.dma_start(out=t_[:],
                                      in_=xT_d.ap()[k * 128:(k + 1) * 128, :].bitcast(F32R))
                    xts.append(t_)

                # ---- V phase: v_aug[t] [128, 8*65], natural [T, vdims] layout ----
                wvs = []
                for k in range(KT):
                    w = p_w.tile([128, 512], F32R, tag="w")
                    nc.sync.dma_start(out=w[:],
                                      in_=wqkv_d.ap()[k * 128:(k + 1) * 128, 1024:1536]
                                      .bitcast(F32R))
                    wvs.append(w)
                if bias:
                    bv = p_tiny.tile([1, 512], F32R, tag="bv")
                    nc.sync.dma_start(out=bv[:], in_=bqkv_d.ap()[1024:1536]
                                      .unsqueeze(0).bitcast(F32R))
                vaugs = []
                for t in range(TT):
                    ps = pp_mm.tile([128, 512], F32, tag="mm")
                    for k in range(KT):
                        nc.tensor.matmul(ps[:], xts[k][:, t * 128:(t + 1) * 128],
                                         wvs[k][:], start=(k == 0),
                                         stop=(not bias and k == KT - 1))
                    if bias:
                        # bias: rank-1 ones x b_v
                        nc.tensor.matmul(ps[:], ones_r[:], bv[:], start=False,
                                         stop=True)
                    va = p_va.tile([128, 8 * 65], F32R, tag="vaug")
                    nc.vector.tensor_copy(
                        va[:].rearrange("p (l c) -> p l c", c=65)[:, :, 0:64],
                        ps[:].rearrange("p (l c) -> p l c", c=64))
                    nc.sync.dma_start(
                        out=va[:].rearrange("p (l c) -> p l c", c=65)[:, :, 64:65],
                        in_=ones_v_d.ap().unsqueeze(2).bitcast(F32R))
                    vaugs.append(va)

                # ---- per head pair: QK projection then causal attention ----
                yts = []
                for hp in range(PAIRS):
                    wqks = []
                    for k in range(KT):
                        wqk = p_w.tile([128, 256], F32R, tag="w", name=f"wqk{hp}_{k}")
                        nc.sync.dma_start(
                            out=wqk[:, 0:128],
                            in_=wqkv_d.ap()[k * 128:(k + 1) * 128,
                                            hp * 128:(hp + 1) * 128].bitcast(F32R))
                        nc.sync.dma_start(
                            out=wqk[:, 128:256],
                            in_=wqkv_d.ap()[k * 128:(k + 1) * 128,
                                            512 + hp * 128:512 + (hp + 1) * 128]
                            .bitcast(F32R))
                        wqks.append(wqk)
                    if bias:
                        bq = p_tiny.tile([128, 1], F32, tag="bq")
                        nc.sync.dma_start(out=bq[:], in_=bqkv_d.ap()
                                          [hp * 128:(hp + 1) * 128].unsqueeze(1))
                        bk = p_tiny.tile([128, 1], F32, tag="bk")
                        nc.sync.dma_start(out=bk[:], in_=bqkv_d.ap()
                                          [512 + hp * 128:512 + (hp + 1) * 128]
                                          .unsqueeze(1))

                    qt = p_qk.tile([128, T], F32R, tag="qt")
                    kt = p_qk.tile([128, T], F32R, tag="kt")
                    for n in range(NQ):
                        sl = slice(n * 512, (n + 1) * 512)
                        psq = pp_mm.tile([128, 512], F32, tag="mm")
                        for k in range(KT):
                            nc.tensor.matmul(psq[:], wqks[k][:, 0:128], xts[k][:, sl],
                                             start=(k == 0), stop=(k == KT - 1))
                        if bias:
                            nc.vector.tensor_scalar_add(qt[:, sl], psq[:], bq[:, 0:1])
                        else:
                            nc.vector.tensor_copy(qt[:, sl], psq[:])
                        psk = pp_mm.tile([128, 512], F32, tag="mm")
                        for k in range(KT):
                            nc.tensor.matmul(psk[:], wqks[k][:, 128:256], xts[k][:, sl],
                                             start=(k == 0), stop=(k == KT - 1))
                        if bias:
                            nc.vector.tensor_scalar_add(kt[:, sl], psk[:], bk[:, 0:1])
                        else:
                            nc.vector.tensor_copy(kt[:, sl], psk[:])

                    # attention for the two heads of this pair
                    yt = p_yt.tile([128, T], F32R, tag="yt")
                    for qb in range(NQ):
                        qsl = slice(qb * 512, (qb + 1) * 512)
                        ots = [pp_ot.tile([65, 512], F32, tag="ot", name=f"ot{hp}_{qb}_{i}")
                               for i in range(2)]
                        ntk = 4 * qb + 4
                        for tk in range(ntk):
                            ksl = slice(tk * 128, (tk + 1) * 128)
                            diag_j = tk - 4 * qb
                            # one double-bank psum holds both heads' S tiles
                            st = pp_st.tile([128, 1024], F32, tag="st")
                            for h01 in range(2):
                                prt = slice(64 * h01, 64 * h01 + 64)
                                nc.tensor.matmul(st[:, 512 * h01:512 * h01 + 512],
                                                 kt[prt, ksl], qt[prt, qsl],
                                                 start=True, stop=True)
                            px = p_px.tile([128, 1024], F32R, tag="pexp")
                            nc.scalar.activation(px[:], st[:], EXP, scale=0.125)
                            if diag_j >= 0:
                                m2 = (masks[diag_j][:].unsqueeze(1)
                                      .broadcast_to([128, 2, 512]))
                                nc.vector.tensor_mul(
                                    px[:].rearrange("p (r f) -> p r f", r=2),
                                    px[:].rearrange("p (r f) -> p r f", r=2), m2)
                            for h01 in range(2):
                                lv = hp * 2 + h01
                                nc.tensor.matmul(ots[h01][:],
                                                 vaugs[tk][:, lv * 65:lv * 65 + 65],
                                                 px[:, 512 * h01:512 * h01 + 512],
                                                 start=(tk == 0),
                                                 stop=(tk == ntk - 1))
                        for h01 in range(2):
                            # copy psum accumulator out promptly to free the bank
                            otc = p_tiny.tile([65, 512], F32R, tag="otc",
                                              name=f"oc{hp}_{qb}_{h01}")
                            nc.vector.tensor_copy(otc[:], ots[h01][:].bitcast(F32R))
                            bcp = pp_ot.tile([64, 512], F32, tag="ot",
                                             name=f"bc{hp}_{qb}_{h01}")
                            nc.tensor.matmul(bcp[:], ones64[64:65, 0:64],
                                             otc[64:65, :], start=True, stop=True)
                            rcp = p_tiny.tile([64, 512], F32, tag="rcp",
                                              name=f"rc{hp}_{qb}_{h01}")
                            nc.vector.reciprocal(rcp[:], bcp[:])
                            nc.vector.tensor_mul(yt[64 * h01:64 * h01 + 64, qsl],
                                                 otc[0:64, :].bitcast(F32), rcp[:])
                    yts.append(yt)

                # ---- projection: out[t, c] = sum_k yt_k[:, t].T @ wp_k ----
                wps = []
                for k in range(PAIRS):
                    w = p_xt.tile([128, C], F32R, tag="xt")
                    nc.sync.dma_start(out=w[:],
                                      in_=wp_d.ap()[k * 128:(k + 1) * 128, :]
                                      .bitcast(F32R))
                    wps.append(w)
                for t in range(TT):
                    for cc in range(2):
                        csl = slice(cc * 512, (cc + 1) * 512)
                        ps = pp_mm.tile([128, 512], F32, tag="mm")
                        for k in range(PAIRS):
                            nc.tensor.matmul(ps[:], yts[k][:, t * 128:(t + 1) * 128],
                                             wps[k][:, csl], start=(k == 0),
                                             stop=(k == PAIRS - 1))
                        ob = p_px.tile([128, 512], F32, tag="pexp")
                        nc.vector.tensor_copy(ob[:], ps[:])
                        nc.sync.dma_start(out=out_d.ap()[t * 128:(t + 1) * 128, csl],
                                          in_=ob[:])
    nc.compile()
    return nc


def _get_nc(bias=False):
    key = ("nc", bias)
    if key not in _NC_CACHE:
        _NC_CACHE[key] = _build(bias=bias)
    return _NC_CACHE[key]


def kernel(x, w_attn, b_attn, w_proj, b_proj):
    x = np.asarray(x, dtype=np.float32)
    w_attn = np.asarray(w_attn, dtype=np.float32)
    b_attn = np.asarray(b_attn, dtype=np.float32)
    w_proj = np.asarray(w_proj, dtype=np.float32)
    b_proj = np.asarray(b_proj, dtype=np.float32)
    nc = _get_nc(bias=bool(np.any(b_attn)))
    in_maps = []
    for c in range(8):
        b, g = divmod(c, 2)
        xT = np.ascontiguousarray(x[b].T)
        s = 512 * g
        wqkv = np.ascontiguousarray(np.concatenate(
            [w_attn[:, s:s + 512],
             w_attn[:, 1024 + s:1024 + s + 512],
             w_attn[:, 2048 + s:2048 + s + 512]], axis=1))
        bqkv = np.ascontiguousarray(np.concatenate(
            [b_attn[s:s + 512], b_attn[1024 + s:1024 + s + 512],
             b_attn[2048 + s:2048 + s + 512]]))
        wp = np.ascontiguousarray(w_proj[s:s + 512, :])
        in_maps.append({"xT": xT, "wqkv": wqkv, "bqkv": bqkv, "wp": wp})
    globals()["_last_in_maps"] = in_maps
    res = run_bass_kernel_spmd(nc, in_maps, list(range(8)))
    out = np.empty((B, T, C), dtype=np.float32)
    for b in range(B):
        out[b] = res.results[2 * b]["out"] + res.results[2 * b + 1]["out"]
    out += b_proj
    return out



# revision 24
# speedup vs baseline: 1.8372x; 1.8372x over previous
"""Causal self-attention (B=4, T=2048, C=1024, 16 heads) on 8 NeuronCores.

Sharding: core c -> batch b=c//2, head group g=c%2 (8 heads each).
Each core computes q,k,v for its 8 heads, causal flash-style attention,
and a partial output projection (row-slice of w_proj). Host sums the two
partials per batch and adds b_proj.

All matmul operands are bf16 (inputs converted on host; intermediates
converted on the PSUM->SBUF copy). PSUM accumulation stays fp32, softmax
normalization math in fp32.

Pipelining: xT is DMA'd in 512-column chunks so the V-phase matmuls start
as soon as the first chunk lands; the output projection is emitted
interleaved into the last head-pair's attention so its matmuls fill the
PE stalls left by the (ACT-bound) exp softmax.

Layout strategy:
  - x is pre-transposed on host -> xT [C, T] so the C-contraction sits on
    SBUF partitions for the qkv projections.
  - q, k are produced transposed (qT/kT [head_dim, T]) which is exactly the
    operand layout for S^T = k q^T tiles [Tk, Tq]; softmax runs along the
    free (Tq) axis per k-row, with row sums obtained for free by appending
    a ones-column to v in the P@V matmul (out row 64 = sum of P columns).
  - v is produced in natural [T, 512] layout (x as stationary operand) and
    stored interleaved per-head with the ones column: [T, 8*65].
  - Normalization: the sums row is broadcast across 64 partitions with a
    K=1 ones matmul, reciprocal'd on DVE, and multiplied into y^T tiles.
  - proj: out[t,c] accumulated over 4 ydim k-tiles (= head pairs).
"""
import numpy as np
import ml_dtypes
import concourse.bass as bass
from concourse import bacc
import concourse.tile as tile
import concourse.mybir as mybir
from concourse.bass_utils import run_bass_kernel_spmd

B, T, C = 4, 2048, 1024
HD = 64            # head dim
HL = 8             # local heads per core
PAIRS = 4          # local head pairs
KT = C // 128      # 8 contraction tiles for qkv
TT = T // 128      # 16 row tiles of T
NQ = T // 512      # 4 query chunks of 512
F32 = mybir.dt.float32
BF16 = mybir.dt.bfloat16
EXP = mybir.ActivationFunctionType.Exp

_NC_CACHE = {}


def _build(nrep=1, bias=False):
    nc = bacc.Bacc("TRN2", target_bir_lowering=False, debug=False)
    xT_d = nc.dram_tensor("xT", [C, T], BF16, kind="ExternalInput")
    wqkv_d = nc.dram_tensor("wqkv", [C, 1536], BF16, kind="ExternalInput")
    bqkv_d = nc.dram_tensor("bqkv", [1536], F32, kind="ExternalInput")
    wp_d = nc.dram_tensor("wp", [512, C], BF16, kind="ExternalInput")
    out_d = nc.dram_tensor("out", [T, C], F32, kind="ExternalOutput")

    # constants baked into the NEFF
    masks_np = np.zeros((4, 128, 512), dtype=np.float32)
    for j in range(4):
        for p in range(128):
            masks_np[j, p, 128 * j + p:] = 1.0
    masks_d = nc.inline_tensor(masks_np.astype(ml_dtypes.bfloat16), name="cmasks")
    ones_r_d = nc.inline_tensor(np.ones((1, 128), dtype=ml_dtypes.bfloat16),
                                name="ones_r")
    ones_v_d = nc.inline_tensor(np.ones((128, 8), dtype=ml_dtypes.bfloat16),
                                name="ones_v")

    with tile.TileContext(nc) as tc:
        with (
            tc.tile_pool(name="xt", bufs=8) as p_xt,        # xT tiles, later wp tiles
            tc.tile_pool(name="w", bufs=8) as p_w,          # w_v then per-pair w_q/w_k
            tc.tile_pool(name="vaug", bufs=16) as p_va,
            tc.tile_pool(name="qk", bufs=2) as p_qk,
            tc.tile_pool(name="yt", bufs=4) as p_yt,
            tc.tile_pool(name="mask", bufs=4) as p_mask,
            tc.tile_pool(name="pexp", bufs=6) as p_px,
            tc.tile_pool(name="tiny", bufs=4) as p_tiny,
            tc.tile_pool(name="mm", bufs=2, space="PSUM") as pp_mm,
            tc.tile_pool(name="st", bufs=2, space="PSUM") as pp_st,
            tc.tile_pool(name="ot", bufs=2, space="PSUM") as pp_ot,
        ):
            # constants: declared here, DMAs issued after the first rep's
            # front-critical loads (they share the ACT hwdge queue)
            ones8 = p_tiny.tile([128, 8], BF16, tag="ones8")
            ones_r = p_tiny.tile([1, 128], BF16, tag="onesr")
            masks = [p_mask.tile([128, 512], BF16, tag="mask", name=f"mask{j}")
                     for j in range(4)]

            def emit_consts():
                nc.scalar.dma_start(out=ones8[:], in_=ones_v_d.ap())
                nc.scalar.dma_start(out=ones_r[:], in_=ones_r_d.ap())
                for j in range(4):
                    nc.scalar.dma_start(out=masks[j][:], in_=masks_d.ap()[j])

            for rep in range(nrep):
                # ---- v weights (SP queue) and xT 1024-col chunks (ACT
                # queue), interleaved by k so the V k-chain starts early ----
                # combined loads: one 3D DMA covers two k-tiles (rows
                # (a p) -> partition p, free block a), halving issue count
                wv2 = [p_w.tile([128, 1024], BF16, tag="w", name=f"wv{rep}_{j}")
                       for j in range(KT // 2)]
                xt2 = [p_xt.tile([128, 2 * T], BF16, tag="xt",
                                 name=f"xt{rep}_{j}") for j in range(KT // 2)]
                wvs = [wv2[k // 2][:, (k % 2) * 512:(k % 2) * 512 + 512]
                       for k in range(KT)]
                xts = [xt2[k // 2][:, (k % 2) * T:(k % 2) * T + T]
                       for k in range(KT)]
                for j in range(KT // 2):
                    nc.sync.dma_start(
                        out=wv2[j][:].rearrange("p (a c) -> p a c", c=512),
                        in_=wqkv_d.ap()[256 * j:256 * (j + 1), 1024:1536]
                        .rearrange("(a p) c -> p a c", p=128))
                    nc.scalar.dma_start(
                        out=xt2[j][:].rearrange("p (a c) -> p a c", c=T)
                        [:, :, 0:1024],
                        in_=xT_d.ap()[256 * j:256 * (j + 1), 0:1024]
                        .rearrange("(a p) c -> p a c", p=128))
                if rep == 0:
                    emit_consts()
                for j in range(KT // 2):
                    nc.scalar.dma_start(
                        out=xt2[j][:].rearrange("p (a c) -> p a c", c=T)
                        [:, :, 1024:2048],
                        in_=xT_d.ap()[256 * j:256 * (j + 1), 1024:2048]
                        .rearrange("(a p) c -> p a c", p=128))
                if bias:
                    bvf = p_tiny.tile([1, 512], F32, tag="bvf")
                    nc.sync.dma_start(out=bvf[:],
                                      in_=bqkv_d.ap()[1024:1536].unsqueeze(0))
                    bv = p_tiny.tile([1, 512], BF16, tag="bv")
                    nc.vector.tensor_copy(bv[:], bvf[:])

                def emit_wqk_dmas(hp):
                    wqk2 = [p_w.tile([128, 512], BF16, tag="w",
                                     name=f"wqk{rep}_{hp}_{j}")
                            for j in range(KT // 2)]
                    for j in range(KT // 2):
                        v = wqk2[j][:].rearrange("p (a c) -> p a c", c=256)
                        nc.sync.dma_start(
                            out=v[:, :, 0:128],
                            in_=wqkv_d.ap()[256 * j:256 * (j + 1),
                                            hp * 128:(hp + 1) * 128]
                            .rearrange("(a p) c -> p a c", p=128))
                        nc.sync.dma_start(
                            out=v[:, :, 128:256],
                            in_=wqkv_d.ap()[256 * j:256 * (j + 1),
                                            512 + hp * 128:512 + (hp + 1) * 128]
                            .rearrange("(a p) c -> p a c", p=128))
                    return [wqk2[k // 2][:, (k % 2) * 256:(k % 2) * 256 + 256]
                            for k in range(KT)]

                wqks0 = emit_wqk_dmas(0)

                # ---- V phase: v_aug[t] [128, 8*65], natural [T, vdims] layout ----
                vaugs = []
                for t in range(TT):
                    if t % 2:
                        ps_t = pp_st.tile([128, 1024], F32, tag="st",
                                          name=f"vps{rep}_{t}")
                        ps = ps_t[:, 0:512]
                    else:
                        ps = pp_mm.tile([128, 512], F32, tag="mm",
                                        name=f"vps{rep}_{t}")
                    for k in range(KT):
                        nc.tensor.matmul(ps[:], xts[k][:, t * 128:(t + 1) * 128],
                                         wvs[k][:], start=(k == 0),
                                         stop=(not bias and k == KT - 1))
                    if bias:
                        # bias: rank-1 ones x b_v
                        nc.tensor.matmul(ps[:], ones_r[:], bv[:], start=False,
                                         stop=True)
                    va = p_va.tile([128, 8 * 65], BF16, tag="vaug")
                    nc.vector.tensor_copy(
                        va[:].rearrange("p (l c) -> p l c", c=65)[:, :, 0:64],
                        ps[:].rearrange("p (l c) -> p l c", c=64))
                    # ones column via DVE (not DMA): a 2-byte DMA scatter at
                    # 130B stride lands on 2B-aligned addresses and can
                    # read-modify-write the neighboring v element.
                    nc.vector.tensor_copy(
                        va[:].rearrange("p (l c) -> p l c", c=65)[:, :, 64:65],
                        ones8[:].unsqueeze(2))
                    vaugs.append(va)

                qk_bias = {}
                if bias:
                    for hp in range(PAIRS):
                        bq = p_tiny.tile([128, 1], F32, tag=f"bq{hp}")
                        nc.sync.dma_start(out=bq[:], in_=bqkv_d.ap()
                                          [hp * 128:(hp + 1) * 128].unsqueeze(1))
                        bk = p_tiny.tile([128, 1], F32, tag=f"bk{hp}")
                        nc.sync.dma_start(out=bk[:], in_=bqkv_d.ap()
                                          [512 + hp * 128:512 + (hp + 1) * 128]
                                          .unsqueeze(1))
                        qk_bias[hp] = (bq, bk)

                def emit_qk_chunk(hp, wqks, qt, kt, n):
                    """One 512-col chunk of the q^T/k^T projections for pair hp."""
                    sl = slice(n * 512, (n + 1) * 512)
                    psq = pp_mm.tile([128, 512], F32, tag="mm")
                    for k in range(KT):
                        nc.tensor.matmul(psq[:], wqks[k][:, 0:128], xts[k][:, sl],
                                         start=(k == 0), stop=(k == KT - 1))
                    if bias:
                        nc.vector.tensor_scalar_add(qt[:, sl], psq[:],
                                                    qk_bias[hp][0][:, 0:1])
                    else:
                        nc.vector.tensor_copy(qt[:, sl], psq[:])
                    psk = pp_mm.tile([128, 512], F32, tag="mm")
                    for k in range(KT):
                        nc.tensor.matmul(psk[:], wqks[k][:, 128:256], xts[k][:, sl],
                                         start=(k == 0), stop=(k == KT - 1))
                    if bias:
                        nc.vector.tensor_scalar_add(kt[:, sl], psk[:],
                                                    qk_bias[hp][1][:, 0:1])
                    else:
                        nc.vector.tensor_copy(kt[:, sl], psk[:])

                def new_qkt(hp):
                    qt = p_qk.tile([128, T], BF16, tag="qt", name=f"qt{rep}_{hp}")
                    kt = p_qk.tile([128, T], BF16, tag="kt", name=f"kt{rep}_{hp}")
                    return qt, kt

                qt0, kt0 = new_qkt(0)
                for n in range(NQ):
                    emit_qk_chunk(0, wqks0, qt0, kt0, n)

                wps = []

                def emit_proj_row(t, yts, tail=False):
                    ob = p_px.tile([128, 1024], F32, tag="pexpf",
                                   name=f"ob{rep}_{t}")
                    for cc in range(2):
                        csl = slice(cc * 512, (cc + 1) * 512)
                        if tail and cc == 1:
                            # attention pools are free in the tail; alternate
                            # psum pools and copy engines to keep the pipe full
                            pst = pp_st.tile([128, 1024], F32, tag="st",
                                             name=f"pjs{rep}_{t}")
                            ps = pst[:, 0:512]
                        else:
                            ps = pp_mm.tile([128, 512], F32, tag="mm",
                                            name=f"pj{rep}_{t}_{cc}")
                        for k in range(PAIRS):
                            nc.tensor.matmul(ps[:],
                                             yts[k][:, t * 128:(t + 1) * 128],
                                             wps[k][:, csl], start=(k == 0),
                                             stop=(k == PAIRS - 1))
                        if tail and cc == 1:
                            nc.scalar.copy(ob[:, csl], ps[:])
                        else:
                            nc.vector.tensor_copy(ob[:, csl], ps[:])
                    (nc.scalar if (tail and t % 2) else nc.sync).dma_start(
                        out=out_d.ap()[t * 128:(t + 1) * 128, :], in_=ob[:])

                # ---- per head pair: causal attention, with the next pair's
                # qk projection (or the output projection) interleaved ----
                yts = []
                cur_qt, cur_kt = qt0, kt0
                cur_wqks = wqks0
                for hp in range(PAIRS):
                    qt, kt = cur_qt, cur_kt
                    if hp < PAIRS - 1:
                        nxt_wqks = emit_wqk_dmas(hp + 1)
                        nxt_qt, nxt_kt = new_qkt(hp + 1)
                    else:
                        wpt = p_xt.tile([128, 4 * C], BF16, tag="xt",
                                        name=f"wp{rep}")
                        nc.sync.dma_start(
                            out=wpt[:].rearrange("p (a c) -> p a c", c=C),
                            in_=wp_d.ap().rearrange("(a p) c -> p a c", p=128))
                        for k in range(PAIRS):
                            wps.append(wpt[:, k * C:(k + 1) * C])

                    yt = p_yt.tile([128, T], BF16, tag="yt")
                    for qb in range(NQ):
                        qsl = slice(qb * 512, (qb + 1) * 512)
                        ots = [pp_ot.tile([65, 512], F32, tag="ot",
                                          name=f"ot{rep}_{hp}_{qb}_{i}")
                               for i in range(2)]
                        ntk = 4 * qb + 4
                        for tk in range(ntk):
                            ksl = slice(tk * 128, (tk + 1) * 128)
                            diag_j = tk - 4 * qb
                            st = pp_st.tile([128, 1024], F32, tag="st")
                            for h01 in range(2):
                                prt = slice(64 * h01, 64 * h01 + 64)
                                nc.tensor.matmul(st[:, 512 * h01:512 * h01 + 512],
                                                 kt[prt, ksl], qt[prt, qsl],
                                                 start=True, stop=True)
                            px = p_px.tile([128, 1024], BF16, tag="pexp")
                            nc.scalar.activation(px[:], st[:], EXP, scale=0.125)
                            if diag_j >= 0:
                                m2 = (masks[diag_j][:].unsqueeze(1)
                                      .broadcast_to([128, 2, 512]))
                                nc.vector.tensor_mul(
                                    px[:].rearrange("p (r f) -> p r f", r=2),
                                    px[:].rearrange("p (r f) -> p r f", r=2), m2)
                            for h01 in range(2):
                                lv = hp * 2 + h01
                                nc.tensor.matmul(ots[h01][:],
                                                 vaugs[tk][:, lv * 65:lv * 65 + 65],
                                                 px[:, 512 * h01:512 * h01 + 512],
                                                 start=(tk == 0),
                                                 stop=(tk == ntk - 1))
                        for h01 in range(2):
                            otc = p_tiny.tile([65, 512], BF16, tag="otc",
                                              name=f"oc{rep}_{hp}_{qb}_{h01}")
                            nc.vector.tensor_copy(otc[:], ots[h01][:])
                            rcp1 = p_tiny.tile([1, 512], BF16, tag="rcp1",
                                               name=f"r1{rep}_{hp}_{qb}_{h01}")
                            with nc.allow_low_precision(
                                    reason="softmax sums bf16 ok"):
                                nc.vector.reciprocal(rcp1[:], otc[64:65, :])
                            rcpb = p_tiny.tile([64, 512], BF16, tag="rcpb",
                                               name=f"rb{rep}_{hp}_{qb}_{h01}")
                            nc.gpsimd.partition_broadcast(rcpb[:], rcp1[:])
                            nc.vector.tensor_mul(yt[64 * h01:64 * h01 + 64, qsl],
                                                 otc[0:64, :], rcpb[:])
                        # filler work for the PE stalls of this (ACT-bound)
                        # attention block: next pair's q/k projection, or for
                        # the last pair the rows of the output projection
                        # whose y tiles are now complete.
                        if hp < PAIRS - 1:
                            emit_qk_chunk(hp + 1, nxt_wqks, nxt_qt, nxt_kt, qb)
                        elif qb > 0:
                            for t in range(4 * (qb - 1), 4 * qb):
                                emit_proj_row(t, yts + [yt])
                    yts.append(yt)
                    if hp < PAIRS - 1:
                        cur_qt, cur_kt = nxt_qt, nxt_kt
                        cur_wqks = nxt_wqks

                # ---- projection tail: last four t rows; attention pools are
                # free now, so alternate psum pools and copy engines to keep
                # the chunk pipeline dense ----
                for t in range(4 * (NQ - 1), TT):
                    emit_proj_row(t, yts, tail=True)
    nc.compile()
    return nc


def _get_nc(bias=False):
    key = ("nc", bias)
    if key not in _NC_CACHE:
        _NC_CACHE[key] = _build(bias=bias)
    return _NC_CACHE[key]


def kernel(x, w_attn, b_attn, w_proj, b_proj):
    x = np.asarray(x, dtype=np.float32)
    w_attn = np.asarray(w_attn, dtype=np.float32)
    b_attn = np.asarray(b_attn, dtype=np.float32)
    w_proj = np.asarray(w_proj, dtype=np.float32)
    b_proj = np.asarray(b_proj, dtype=np.float32)
    nc = _get_nc(bias=bool(np.any(b_attn)))
    in_maps = []
    for c in range(8):
        b, g = divmod(c, 2)
        xT = np.ascontiguousarray(x[b].T).astype(ml_dtypes.bfloat16)
        s = 512 * g
        wqkv = np.ascontiguousarray(np.concatenate(
            [w_attn[:, s:s + 512],
             w_attn[:, 1024 + s:1024 + s + 512],
             w_attn[:, 2048 + s:2048 + s + 512]], axis=1)).astype(ml_dtypes.bfloat16)
        bqkv = np.ascontiguousarray(np.concatenate(
            [b_attn[s:s + 512], b_attn[1024 + s:1024 + s + 512],
             b_attn[2048 + s:2048 + s + 512]]))
        wp = np.ascontiguousarray(w_proj[s:s + 512, :]).astype(ml_dtypes.bfloat16)
        in_maps.append({"xT": xT, "wqkv": wqkv, "bqkv": bqkv, "wp": wp})
    globals()["_last_in_maps"] = in_maps
    res = run_bass_kernel_spmd(nc, in_maps, list(range(8)))
    out = np.empty((B, T, C), dtype=np.float32)
    for b in range(B):
        out[b] = res.results[2 * b]["out"] + res.results[2 * b + 1]["out"]
    out += b_proj
    return out
